# revision 12
# baseline (speedup 1.0000x reference)
"""Causal single-head attention (B=4, S=4096, D=1024) on 8 TRN2 NeuronCores.

Sharding: core = (batch b, half h).  Each core computes attention output for
2048 queries of one batch: query chunks {0,3,4,7} (h=0) or {1,2,5,6} (h=1) of
8x512, which balances causal work.  K^T is kept fp16 SBUF-resident; V goes to
a DRAM scratch (fp16) and is streamed per k-tile.  Scores are computed in the
S^T = [k, q] layout so no on-device transposes are needed anywhere:
  KT/QT/V projections:  psum = sum_d WT[d,:128].T @ xT[d,:]   (fp32r)
  scores^T[k,q]       :  psum = sum_o KT[o,k128].T @ QT[o,q512]  (fp16)
  P = exp(s*scale) * causal_mask   (mask = (iota_k - iota_q) <= a[slot,j])
  den[1,q]            :  ones[k,1].T @ P^T                      (fp16)
  ctx^T[o,q]          :  psum = sum_k V[k,o128].T @ P^T[k,q]    (fp16)
  out = ctx^T * (1/den)  broadcast via ones[1,128].T @ recip[1,q] outer product
"""

import sys

for _p in ("/opt/trn_rl_repo",):
    if _p not in sys.path:
        sys.path.insert(0, _p)

import numpy as np

B, S, D = 4, 4096, 1024
P = 128
CH = 512                       # query chunk
NSLOT = 4                      # chunks per core
NQ = NSLOT * CH                # queries per core
NK = [8, 16, 24, 32]           # k-tiles per slot (uniform across cores)
SLOTBASE = [0, 8, 24, 48]      # amat column base per slot
CHUNKS_H = [[0, 3, 4, 7], [1, 2, 5, 6]]
SCALE = 1.0 / 32.0             # 1/sqrt(D)

_PROGRAM = None


def _build_program():
    import concourse.bass as bass
    import concourse.tile as tile
    import concourse.mybir as mybir
    from concourse import bacc
    from concourse.bass import ds, ts

    f32 = mybir.dt.float32
    f32r = mybir.dt.float32r
    f16 = mybir.dt.float16

    nc = bacc.Bacc(trn_type="TRN2", target_bir_lowering=False, debug=False, num_devices=8)

    xkv = nc.declare_dram_parameter("xkv", [D, S // 2], f16, isOutput=False)
    xqT = nc.declare_dram_parameter("xqT", [D, NQ], f16, isOutput=False)
    wqT = nc.declare_dram_parameter("wqT", [D, D], f16, isOutput=False)
    wkT = nc.declare_dram_parameter("wkT", [D, D], f16, isOutput=False)
    wvT = nc.declare_dram_parameter("wvT", [D, D], f16, isOutput=False)
    amat = nc.declare_dram_parameter("amat", [P, 80], f32, isOutput=False)
    dmat = nc.declare_dram_parameter("dmat", [P, CH], f32, isOutput=False)
    ones_k = nc.declare_dram_parameter("ones_k", [P, 1], f16, isOutput=False)
    ones_r = nc.declare_dram_parameter("ones_r", [1, P], f32, isOutput=False)
    outT = nc.declare_dram_parameter("outT", [D, NQ], f32, isOutput=True)

    kt_partA = nc.dram_tensor("kt_partA", [D, S // 4], f16)
    kt_partB = nc.dram_tensor("kt_partB", [D, S // 4], f16)
    v_partA = nc.dram_tensor("v_partA", [S // 4, D], f16)
    v_partB = nc.dram_tensor("v_partB", [S // 4, D], f16)
    kt_fullA = nc.dram_tensor("kt_fullA", [2 * D, S // 4], f16)
    kt_fullB = nc.dram_tensor("kt_fullB", [2 * D, S // 4], f16)
    vscrA = nc.dram_tensor("v_scratchA", [S // 2, D], f16)
    vscrB = nc.dram_tensor("v_scratchB", [S // 2, D], f16)
    PAIRS = [[0, 1], [2, 3], [4, 5], [6, 7]]

    Exp = mybir.ActivationFunctionType.Exp
    is_le = mybir.AluOpType.is_le
    mult = mybir.AluOpType.mult

    with tile.TileContext(nc, pool_alloc_mode="queue") as tc:
        with (
            tc.tile_pool(name="kt", bufs=1) as kt_pool,
            tc.tile_pool(name="qt", bufs=1) as qt_pool,
            tc.tile_pool(name="const", bufs=1) as const_pool,
        ):
            KTp = [
                kt_pool.tile([P, 8, S // 4], f16, tag=f"kt{i}", name=f"KTp{i}")
                for i in range(4)
            ]
            QT = qt_pool.tile([P, 8, NQ], f16)
            dmat_sb = const_pool.tile([P, CH], f32, tag="dmat")
            amat_sb = const_pool.tile([P, 80], f32, tag="amat")
            ones_k_sb = const_pool.tile([P, 1], f16, tag="onesk")
            ones_r_sb = const_pool.tile([1, P], f32, tag="onesr")
            nc.sync.dma_start(out=dmat_sb[:], in_=dmat[:])
            nc.sync.dma_start(out=amat_sb[:], in_=amat[:])
            nc.sync.dma_start(out=ones_k_sb[:], in_=ones_k[:])
            nc.sync.dma_start(out=ones_r_sb[:], in_=ones_r[:])

            # ---------------- Phase 0: project own half of K^T and V -------
            with (
                tc.tile_pool(name="w0", bufs=1) as w_pool,
                tc.tile_pool(name="xc", bufs=1) as x_pool,
                tc.tile_pool(name="vb", bufs=3) as vb_pool,
                tc.tile_pool(name="kb", bufs=3) as kb_pool,
                tc.tile_pool(name="ps0", bufs=4, space="PSUM") as ps_pool,
            ):
                wk = w_pool.tile([P, 8, D], f16, tag="wk")
                wv = w_pool.tile([P, 8, D], f16, tag="wv")
                nc.sync.dma_start(
                    out=wk[:], in_=wkT[:].rearrange("(a p) o -> p a o", p=P)
                )
                nc.sync.dma_start(
                    out=wv[:], in_=wvT[:].rearrange("(a p) o -> p a o", p=P)
                )
                xcs = []
                for c in range(4):
                    xc = x_pool.tile([P, 8, CH], f16, tag=f"xc{c}", name=f"xc{c}")
                    nc.sync.dma_start(
                        out=xc[:],
                        in_=xkv[:, ts(c, CH)].rearrange("(a p) s -> p a s", p=P),
                    )
                    xcs.append(xc)

                def proj_k(c):
                    for o in range(8):
                        ps = ps_pool.tile([P, CH], f32, tag="ps")
                        for d in range(8):
                            nc.tensor.matmul(
                                ps[:],
                                lhsT=wk[:, d, ts(o, P)],
                                rhs=xcs[c][:, d, :],
                                start=(d == 0),
                                stop=(d == 7),
                            )
                        kb = kb_pool.tile([P, CH], f16, tag="kb")
                        nc.vector.tensor_copy(kb[:], ps[:])
                        ktp = kt_partA if c < 2 else kt_partB
                        nc.sync.dma_start(
                            out=ktp[ds(o * P, P), ts(c % 2, CH)], in_=kb[:]
                        )

                def proj_v(c):
                    for kt_i in range(4):
                        vb = vb_pool.tile([P, D], f16, tag="vb")
                        for oh in range(2):
                            ps = ps_pool.tile([P, CH], f32, tag="ps")
                            for d in range(8):
                                nc.tensor.matmul(
                                    ps[:],
                                    lhsT=xcs[c][:, d, ts(kt_i, P)],
                                    rhs=wv[:, d, ts(oh, CH)],
                                    start=(d == 0),
                                    stop=(d == 7),
                                )
                            nc.vector.tensor_copy(vb[:, ts(oh, CH)], ps[:])
                        vp = v_partA if c < 2 else v_partB
                        nc.sync.dma_start(
                            out=vp[ds((c % 2) * CH + kt_i * P, P), :], in_=vb[:]
                        )

                def gather(kind_ins, kind_outs):
                    nc.gpsimd.collective_compute(
                        "AllGather",
                        mybir.AluOpType.bypass,
                        replica_groups=PAIRS,
                        ins=[kind_ins],
                        outs=[kind_outs],
                    )

                proj_k(0)
                proj_k(1)
                gather(kt_partA[:], kt_fullA[:])
                for r in range(2):
                    nc.scalar.dma_start(
                        out=KTp[r * 2][:],
                        in_=kt_fullA[ds(r * D, D), :].rearrange(
                            "(a p) k -> p a k", p=P
                        ),
                    )
                proj_v(0)
                proj_v(1)
                gather(v_partA[:], vscrA[:])
                proj_k(2)
                proj_k(3)
                gather(kt_partB[:], kt_fullB[:])
                for r in range(2):
                    nc.scalar.dma_start(
                        out=KTp[r * 2 + 1][:],
                        in_=kt_fullB[ds(r * D, D), :].rearrange(
                            "(a p) k -> p a k", p=P
                        ),
                    )
                proj_v(2)
                proj_v(3)
                gather(v_partB[:], vscrB[:])

            # ---------------- Phase 1: Q^T (resident) ----------------------
            with (
                tc.tile_pool(name="w1", bufs=1) as w1_pool,
                tc.tile_pool(name="xq", bufs=2) as xq_pool,
                tc.tile_pool(name="ps1", bufs=4, space="PSUM") as ps1_pool,
            ):
                wq = w1_pool.tile([P, 8, D], f16, tag="wq")
                nc.sync.dma_start(
                    out=wq[:], in_=wqT[:].rearrange("(a p) o -> p a o", p=P)
                )
                for c in range(NSLOT):
                    xq = xq_pool.tile([P, 8, CH], f16)
                    nc.sync.dma_start(
                        out=xq[:],
                        in_=xqT[:, ts(c, CH)].rearrange("(a p) s -> p a s", p=P),
                    )
                    for o in range(8):
                        ps = ps1_pool.tile([P, CH], f32)
                        for d in range(8):
                            nc.tensor.matmul(
                                ps[:],
                                lhsT=wq[:, d, ts(o, P)],
                                rhs=xq[:, d, :],
                                start=(d == 0),
                                stop=(d == 7),
                            )
                        nc.vector.tensor_copy(QT[:, o, ts(c, CH)], ps[:])

            # ---------------- Phase 2: attention ---------------------------
            with (
                tc.tile_pool(name="ctx", bufs=2) as ctx_pool,
                tc.tile_pool(name="vt", bufs=8) as v_pool,
                tc.tile_pool(name="pt", bufs=10) as p_pool,
                tc.tile_pool(name="et", bufs=3) as e_pool,
                tc.tile_pool(name="fo", bufs=3) as f_pool,
                tc.tile_pool(name="dsb", bufs=2) as den_pool,
                tc.tile_pool(name="pss", bufs=3, space="PSUM") as s_ps_pool,
                tc.tile_pool(name="psc", bufs=2, space="PSUM") as c_ps_pool,
                tc.tile_pool(name="psd", bufs=2, space="PSUM") as d_ps_pool,
                tc.tile_pool(name="psb", bufs=1, space="PSUM") as b_ps_pool,
            ):
                for slot in range(NSLOT):
                    nk = NK[slot]
                    ctx = ctx_pool.tile([P, 8, CH], f32, tag="ctx")
                    den = den_pool.tile([1, CH], f32, tag="den")
                    for blk in range(nk // 4):
                        p_tiles = []
                        v_tiles = []
                        for j4 in range(4):
                            j = blk * 4 + j4
                            vt = v_pool.tile([P, D], f16, tag="vt")
                            q, rr = divmod(j, 8)
                            vbuf = [vscrA, vscrB, vscrA, vscrB][q]
                            vrow = (q // 2) * 1024 + rr * P
                            nc.scalar.dma_start(out=vt[:], in_=vbuf[ds(vrow, P), :])
                            sps = s_ps_pool.tile([P, CH], f32)
                            for o in range(8):
                                nc.tensor.matmul(
                                    sps[:],
                                    lhsT=KTp[j // 8][:, o, ds((j % 8) * P, P)],
                                    rhs=QT[:, o, ts(slot, CH)],
                                    start=(o == 0),
                                    stop=(o == 7),
                                )
                            et = e_pool.tile([P, CH], f32, tag="et")
                            nc.scalar.activation(et[:], sps[:], Exp, scale=SCALE)
                            pt = p_pool.tile([P, CH], f16, tag="pt")
                            col = SLOTBASE[slot] + j
                            nc.vector.scalar_tensor_tensor(
                                out=pt[:],
                                in0=dmat_sb[:],
                                scalar=amat_sb[:, ds(col, 1)],
                                in1=et[:],
                                op0=is_le,
                                op1=mult,
                            )
                            p_tiles.append(pt)
                            v_tiles.append(vt)
                        dps = d_ps_pool.tile([1, CH], f32)
                        for j4 in range(4):
                            nc.tensor.matmul(
                                dps[:],
                                lhsT=ones_k_sb[:],
                                rhs=p_tiles[j4][:],
                                start=(j4 == 0),
                                stop=(j4 == 3),
                            )
                        if blk == 0:
                            nc.vector.tensor_copy(den[:], dps[:])
                        else:
                            nc.vector.tensor_add(den[:], den[:], dps[:])
                        for o in range(8):
                            cps = c_ps_pool.tile([P, CH], f32)
                            for j4 in range(4):
                                nc.tensor.matmul(
                                    cps[:],
                                    lhsT=v_tiles[j4][:, ts(o, P)],
                                    rhs=p_tiles[j4][:],
                                    start=(j4 == 0),
                                    stop=(j4 == 3),
                                )
                            if blk == 0:
                                nc.vector.tensor_copy(ctx[:, o, :], cps[:])
                            else:
                                nc.vector.tensor_add(ctx[:, o, :], ctx[:, o, :], cps[:])
                    rec = den_pool.tile([1, CH], f32, tag="rec")
                    nc.vector.reciprocal(rec[:], den[:])
                    bps = b_ps_pool.tile([P, CH], f32)
                    nc.tensor.matmul(
                        bps[:], lhsT=ones_r_sb[:], rhs=rec[:], start=True, stop=True
                    )
                    for o in range(8):
                        ft = f_pool.tile([P, CH], f32, tag="ft")
                        nc.vector.tensor_mul(ft[:], ctx[:, o, :], bps[:])
                        nc.sync.dma_start(
                            out=outT[ds(o * P, P), ts(slot, CH)], in_=ft[:]
                        )

    nc.compile()
    return nc


def _get_program():
    global _PROGRAM
    if _PROGRAM is None:
        _PROGRAM = _build_program()
    return _PROGRAM


def _make_in_maps(x, W_query, W_key, W_value):
    xT = np.ascontiguousarray(
        np.asarray(x, dtype=np.float32).transpose(0, 2, 1).astype(np.float16)
    )
    wqT = np.ascontiguousarray(np.asarray(W_query, dtype=np.float32).T.astype(np.float16))
    wkT = np.ascontiguousarray(np.asarray(W_key, dtype=np.float32).T.astype(np.float16))
    wvT = np.ascontiguousarray(np.asarray(W_value, dtype=np.float32).T.astype(np.float16))
    dmat = (
        np.arange(P, dtype=np.float32)[:, None] - np.arange(CH, dtype=np.float32)[None, :]
    )
    dmat = np.ascontiguousarray(dmat)
    amat_h = []
    for h in range(2):
        a = np.zeros((P, 80), np.float32)
        for slot in range(NSLOT):
            cid = CHUNKS_H[h][slot]
            for j in range(NK[slot]):
                a[:, SLOTBASE[slot] + j] = CH * cid - P * j
        amat_h.append(a)
    ones_k = np.ones((P, 1), np.float16)
    ones_r = np.ones((1, P), np.float32)

    in_maps = []
    for core in range(8):
        b, h = core // 2, core % 2
        xq_cols = np.concatenate(
            [np.arange(c * CH, (c + 1) * CH) for c in CHUNKS_H[h]]
        )
        xqT_b = np.ascontiguousarray(xT[b][:, xq_cols])
        xkv_b = np.ascontiguousarray(xT[b][:, h * (S // 2) : (h + 1) * (S // 2)])
        in_maps.append(
            {
                "xkv": xkv_b,
                "xqT": xqT_b,
                "wqT": wqT,
                "wkT": wkT,
                "wvT": wvT,
                "amat": amat_h[h],
                "dmat": dmat,
                "ones_k": ones_k,
                "ones_r": ones_r,
            }
        )
    return in_maps


def _assemble(results):
    out = np.empty((B, S, D), np.float32)
    for core in range(8):
        b, h = core // 2, core % 2
        oT = np.asarray(results[core]["outT"])  # [D, NQ]
        for slot, c in enumerate(CHUNKS_H[h]):
            out[b, c * CH : (c + 1) * CH, :] = oT[:, slot * CH : (slot + 1) * CH].T
    return out


def run(inputs, trace=False, trace_cores=None):
    """Run the kernel; returns (output, BassKernelResults)."""
    from concourse.bass_utils import run_bass_kernel_spmd

    nc = _get_program()
    in_maps = _make_in_maps(
        inputs["x"], inputs["W_query"], inputs["W_key"], inputs["W_value"]
    )
    kw = {}
    if trace:
        kw = dict(trace=True, trace_cores=trace_cores, stitch_traces=False)
    res = run_bass_kernel_spmd(nc, in_maps, list(range(8)), **kw)
    return _assemble(res.results), res


def kernel(x, W_query, W_key, W_value):
    out, _ = run({"x": x, "W_query": W_query, "W_key": W_key, "W_value": W_value})
    return out


# revision 13
# speedup vs baseline: 1.0288x; 1.0288x over previous
"""Causal single-head attention (B=4, S=4096, D=1024) on 8 TRN2 NeuronCores.

Sharding: core = (batch b, half h).  Each core computes attention output for
2048 queries of one batch: query chunks {0,3,4,7} (h=0) or {1,2,5,6} (h=1) of
8x512, which balances causal work.  The K/V projection is split across the
core pair (each projects its half of the sequence from the host-sliced xkv)
and assembled with pairwise AllGathers, chunked and interleaved with compute
so the exchange hides behind the projections.  K^T lives in SBUF as four
independently-gated tiles; V is streamed from the gathered DRAM scratch.
Scores are computed in the S^T = [k, q] layout so no on-device transposes are
needed anywhere:
  K^T/Q^T/V projections:  psum = sum_d WT[d,:128].T @ x^T[d,:]      (fp16)
  scores^T[k,q]        :  psum = sum_o KT[o,k128].T @ QT[o,q512]    (fp16)
  P = exp(s*scale) * causal_mask   (mask = (iota_k - iota_q) <= a[slot,j])
  den[1,q]             :  ones[k,1].T @ P^T                         (fp16)
  ctx^T[o,q]           :  psum = sum_k V[k,o128].T @ P^T[k,q]       (fp16)
  out = ctx^T * (1/den)  broadcast via ones[1,128].T @ recip[1,q]
"""

import sys

for _p in ("/opt/trn_rl_repo",):
    if _p not in sys.path:
        sys.path.insert(0, _p)

import numpy as np

B, S, D = 4, 4096, 1024
P = 128
CH = 512                       # query chunk
NSLOT = 4                      # chunks per core
NQ = NSLOT * CH                # queries per core
NK = [8, 16, 24, 32]           # k-tiles per slot (uniform across cores)
SLOTBASE = [0, 8, 24, 48]      # amat column base per slot
CHUNKS_H = [[0, 3, 4, 7], [1, 2, 5, 6]]
SCALE = 1.0 / 32.0             # 1/sqrt(D)

_PROGRAM = None


def _build_program():
    import concourse.bass as bass
    import concourse.tile as tile
    import concourse.mybir as mybir
    from concourse import bacc
    from concourse.bass import ds, ts

    f32 = mybir.dt.float32
    f16 = mybir.dt.float16

    nc = bacc.Bacc(trn_type="TRN2", target_bir_lowering=False, debug=False,
                   num_devices=8)

    xkv = nc.declare_dram_parameter("xkv", [D, S // 2], f16, isOutput=False)
    xqT = nc.declare_dram_parameter("xqT", [D, NQ], f16, isOutput=False)
    wqT = nc.declare_dram_parameter("wqT", [D, D], f16, isOutput=False)
    wkT = nc.declare_dram_parameter("wkT", [D, D], f16, isOutput=False)
    wvT = nc.declare_dram_parameter("wvT", [D, D], f16, isOutput=False)
    amat = nc.declare_dram_parameter("amat", [P, 80], f32, isOutput=False)
    dmat = nc.declare_dram_parameter("dmat", [P, CH], f32, isOutput=False)
    ones_k = nc.declare_dram_parameter("ones_k", [P, 1], f16, isOutput=False)
    ones_r = nc.declare_dram_parameter("ones_r", [1, P], f32, isOutput=False)
    outT = nc.declare_dram_parameter("outT", [D, NQ], f32, isOutput=True)

    H = S // 4  # 1024: columns per K/V exchange piece
    kt_partA = nc.dram_tensor("kt_partA", [D, H], f16)
    kt_partB = nc.dram_tensor("kt_partB", [D, H], f16)
    v_partA = nc.dram_tensor("v_partA", [H, D], f16)
    v_partB = nc.dram_tensor("v_partB", [H, D], f16)
    kt_fullA = nc.dram_tensor("kt_fullA", [2 * D, H], f16)
    kt_fullB = nc.dram_tensor("kt_fullB", [2 * D, H], f16)
    vscrA = nc.dram_tensor("v_scratchA", [2 * H, D], f16)
    vscrB = nc.dram_tensor("v_scratchB", [2 * H, D], f16)
    PAIRS = [[0, 1], [2, 3], [4, 5], [6, 7]]

    Exp = mybir.ActivationFunctionType.Exp
    is_le = mybir.AluOpType.is_le
    mult = mybir.AluOpType.mult

    with tile.TileContext(nc, pool_alloc_mode="queue") as tc:
        with (
            tc.tile_pool(name="kt", bufs=1) as kt_pool,
            tc.tile_pool(name="qt", bufs=1) as qt_pool,
            tc.tile_pool(name="const", bufs=1) as const_pool,
        ):
            KTp = [
                kt_pool.tile([P, 8, H], f16, tag=f"kt{i}", name=f"KTp{i}")
                for i in range(4)
            ]
            QTs = [
                qt_pool.tile([P, 8, CH], f16, tag=f"qt{i}", name=f"QTs{i}")
                for i in range(NSLOT)
            ]
            dmat_sb = const_pool.tile([P, CH], f32, tag="dmat")
            amat_sb = const_pool.tile([P, 80], f32, tag="amat")
            ones_k_sb = const_pool.tile([P, 1], f16, tag="onesk")
            ones_r_sb = const_pool.tile([1, P], f32, tag="onesr")
            nc.sync.dma_start(out=dmat_sb[:], in_=dmat[:])
            nc.sync.dma_start(out=amat_sb[:], in_=amat[:])
            nc.sync.dma_start(out=ones_k_sb[:], in_=ones_k[:])
            nc.sync.dma_start(out=ones_r_sb[:], in_=ones_r[:])

            # ---------- Phase 0+1: projections with pipelined exchange ------
            with (
                tc.tile_pool(name="w0", bufs=1) as w_pool,
                tc.tile_pool(name="xc", bufs=1) as x_pool,
                tc.tile_pool(name="xq", bufs=2) as xq_pool,
                tc.tile_pool(name="vb", bufs=2) as vb_pool,
                tc.tile_pool(name="kb", bufs=2) as kb_pool,
                tc.tile_pool(name="ps0", bufs=4, space="PSUM") as ps_pool,
            ):
                wk = w_pool.tile([P, 8, D], f16, tag="wk")
                wv = w_pool.tile([P, 8, D], f16, tag="wv")
                wq = w_pool.tile([P, 8, D], f16, tag="wq")
                # SP-queue loads: K/V weights + x chunks, issued immediately
                nc.sync.dma_start(
                    out=wk[:], in_=wkT[:].rearrange("(a p) o -> p a o", p=P)
                )
                nc.sync.dma_start(
                    out=wv[:], in_=wvT[:].rearrange("(a p) o -> p a o", p=P)
                )
                xcs = []
                for c in range(4):
                    xc = x_pool.tile([P, 8, CH], f16, tag=f"xc{c}", name=f"xc{c}")
                    nc.sync.dma_start(
                        out=xc[:],
                        in_=xkv[:, ts(c, CH)].rearrange("(a p) s -> p a s", p=P),
                    )
                    xcs.append(xc)
                # ACT-queue loads: Q-side inputs, issued immediately
                nc.scalar.dma_start(
                    out=wq[:], in_=wqT[:].rearrange("(a p) o -> p a o", p=P)
                )

                def load_xq(c):
                    xq = xq_pool.tile([P, 8, CH], f16, tag="xq", name=f"xq{c}")
                    nc.scalar.dma_start(
                        out=xq[:],
                        in_=xqT[:, ts(c, CH)].rearrange("(a p) s -> p a s", p=P),
                    )
                    return xq

                xq_pending = [load_xq(0), load_xq(1)]

                def proj_k(c):
                    for o in range(8):
                        ps = ps_pool.tile([P, CH], f32, tag="ps", name="psk")
                        for d in range(8):
                            nc.tensor.matmul(
                                ps[:],
                                lhsT=wk[:, d, ts(o, P)],
                                rhs=xcs[c][:, d, :],
                                start=(d == 0),
                                stop=(d == 7),
                            )
                        kb = kb_pool.tile([P, CH], f16, tag="kb", name="kb")
                        nc.vector.tensor_copy(kb[:], ps[:])
                        ktp = kt_partA if c < 2 else kt_partB
                        nc.sync.dma_start(
                            out=ktp[ds(o * P, P), ts(c % 2, CH)], in_=kb[:]
                        )

                def proj_v(c):
                    for kt_i in range(4):
                        vb = vb_pool.tile([P, D], f16, tag="vb", name="vb")
                        for oh in range(2):
                            ps = ps_pool.tile([P, CH], f32, tag="ps", name="psv")
                            for d in range(8):
                                nc.tensor.matmul(
                                    ps[:],
                                    lhsT=xcs[c][:, d, ts(kt_i, P)],
                                    rhs=wv[:, d, ts(oh, CH)],
                                    start=(d == 0),
                                    stop=(d == 7),
                                )
                            nc.vector.tensor_copy(vb[:, ts(oh, CH)], ps[:])
                        vp = v_partA if c < 2 else v_partB
                        nc.sync.dma_start(
                            out=vp[ds((c % 2) * CH + kt_i * P, P), :], in_=vb[:]
                        )

                def proj_q(slot):
                    xq = xq_pending[slot]
                    for o in range(8):
                        ps = ps_pool.tile([P, CH], f32, tag="ps", name="psq")
                        for d in range(8):
                            nc.tensor.matmul(
                                ps[:],
                                lhsT=wq[:, d, ts(o, P)],
                                rhs=xq[:, d, :],
                                start=(d == 0),
                                stop=(d == 7),
                            )
                        nc.vector.tensor_copy(QTs[slot][:, o, :], ps[:])

                def gather(src, dst):
                    nc.gpsimd.collective_compute(
                        "AllGather",
                        mybir.AluOpType.bypass,
                        replica_groups=PAIRS,
                        ins=[src[:]],
                        outs=[dst[:]],
                    )

                def load_ktp(full, base):
                    for r in range(2):
                        nc.scalar.dma_start(
                            out=KTp[r * 2 + base][:],
                            in_=full[ds(r * D, D), :].rearrange(
                                "(a p) k -> p a k", p=P
                            ),
                        )

                proj_k(0)
                proj_k(1)
                gather(kt_partA, kt_fullA)
                load_ktp(kt_fullA, 0)
                proj_q(0)
                xq_pending.append(load_xq(2))
                proj_k(2)
                proj_k(3)
                gather(kt_partB, kt_fullB)
                load_ktp(kt_fullB, 1)
                proj_q(1)
                xq_pending.append(load_xq(3))
                proj_v(0)
                proj_v(1)
                gather(v_partA, vscrA)
                proj_q(2)
                proj_q(3)
                proj_v(2)
                proj_v(3)
                gather(v_partB, vscrB)

            # ---------------- Phase 2: attention ---------------------------
            with (
                tc.tile_pool(name="ctx", bufs=2) as ctx_pool,
                tc.tile_pool(name="vt", bufs=8) as v_pool,
                tc.tile_pool(name="pt", bufs=10) as p_pool,
                tc.tile_pool(name="et", bufs=3) as e_pool,
                tc.tile_pool(name="fo", bufs=3) as f_pool,
                tc.tile_pool(name="dsb", bufs=2) as den_pool,
                tc.tile_pool(name="pss", bufs=3, space="PSUM") as s_ps_pool,
                tc.tile_pool(name="psc", bufs=2, space="PSUM") as c_ps_pool,
                tc.tile_pool(name="psd", bufs=2, space="PSUM") as d_ps_pool,
                tc.tile_pool(name="psb", bufs=1, space="PSUM") as b_ps_pool,
            ):
                for slot in range(NSLOT):
                    nk = NK[slot]
                    ctx = ctx_pool.tile([P, 8, CH], f32, tag="ctx", name="ctx")
                    den = den_pool.tile([1, CH], f32, tag="den", name="den")
                    for blk in range(nk // 4):
                        p_tiles = []
                        v_tiles = []
                        for j4 in range(4):
                            j = blk * 4 + j4
                            vt = v_pool.tile([P, D], f16, tag="vt", name="vt")
                            q, rr = divmod(j, 8)
                            vbuf = [vscrA, vscrB, vscrA, vscrB][q]
                            vrow = (q // 2) * H + rr * P
                            nc.scalar.dma_start(out=vt[:], in_=vbuf[ds(vrow, P), :])
                            sps = s_ps_pool.tile([P, CH], f32, name="sps")
                            for o in range(8):
                                nc.tensor.matmul(
                                    sps[:],
                                    lhsT=KTp[j // 8][:, o, ds((j % 8) * P, P)],
                                    rhs=QTs[slot][:, o, :],
                                    start=(o == 0),
                                    stop=(o == 7),
                                )
                            et = e_pool.tile([P, CH], f32, tag="et", name="et")
                            nc.scalar.activation(et[:], sps[:], Exp, scale=SCALE)
                            pt = p_pool.tile([P, CH], f16, tag="pt", name="pt")
                            col = SLOTBASE[slot] + j
                            nc.vector.scalar_tensor_tensor(
                                out=pt[:],
                                in0=dmat_sb[:],
                                scalar=amat_sb[:, ds(col, 1)],
                                in1=et[:],
                                op0=is_le,
                                op1=mult,
                            )
                            p_tiles.append(pt)
                            v_tiles.append(vt)
                        dps = d_ps_pool.tile([1, CH], f32, name="dps")
                        for j4 in range(4):
                            nc.tensor.matmul(
                                dps[:],
                                lhsT=ones_k_sb[:],
                                rhs=p_tiles[j4][:],
                                start=(j4 == 0),
                                stop=(j4 == 3),
                            )
                        if blk == 0:
                            nc.vector.tensor_copy(den[:], dps[:])
                        else:
                            nc.vector.tensor_add(den[:], den[:], dps[:])
                        for o in range(8):
                            cps = c_ps_pool.tile([P, CH], f32, name="cps")
                            for j4 in range(4):
                                nc.tensor.matmul(
                                    cps[:],
                                    lhsT=v_tiles[j4][:, ts(o, P)],
                                    rhs=p_tiles[j4][:],
                                    start=(j4 == 0),
                                    stop=(j4 == 3),
                                )
                            if blk == 0:
                                nc.vector.tensor_copy(ctx[:, o, :], cps[:])
                            else:
                                nc.vector.tensor_add(
                                    ctx[:, o, :], ctx[:, o, :], cps[:]
                                )
                    rec = den_pool.tile([1, CH], f32, tag="rec", name="rec")
                    nc.vector.reciprocal(rec[:], den[:])
                    bps = b_ps_pool.tile([P, CH], f32, name="bps")
                    nc.tensor.matmul(
                        bps[:], lhsT=ones_r_sb[:], rhs=rec[:], start=True, stop=True
                    )
                    for o in range(8):
                        ft = f_pool.tile([P, CH], f32, tag="ft", name="ft")
                        nc.vector.tensor_mul(ft[:], ctx[:, o, :], bps[:])
                        nc.sync.dma_start(
                            out=outT[ds(o * P, P), ts(slot, CH)], in_=ft[:]
                        )

    nc.compile()
    return nc


def _get_program():
    global _PROGRAM
    if _PROGRAM is None:
        _PROGRAM = _build_program()
    return _PROGRAM


def _make_in_maps(x, W_query, W_key, W_value):
    xT = np.ascontiguousarray(
        np.asarray(x, dtype=np.float32).transpose(0, 2, 1).astype(np.float16)
    )
    wqT = np.ascontiguousarray(np.asarray(W_query, dtype=np.float32).T.astype(np.float16))
    wkT = np.ascontiguousarray(np.asarray(W_key, dtype=np.float32).T.astype(np.float16))
    wvT = np.ascontiguousarray(np.asarray(W_value, dtype=np.float32).T.astype(np.float16))
    dmat = (
        np.arange(P, dtype=np.float32)[:, None] - np.arange(CH, dtype=np.float32)[None, :]
    )
    dmat = np.ascontiguousarray(dmat)
    amat_h = []
    for h in range(2):
        a = np.zeros((P, 80), np.float32)
        for slot in range(NSLOT):
            cid = CHUNKS_H[h][slot]
            for j in range(NK[slot]):
                a[:, SLOTBASE[slot] + j] = CH * cid - P * j
        amat_h.append(a)
    ones_k = np.ones((P, 1), np.float16)
    ones_r = np.ones((1, P), np.float32)

    in_maps = []
    for core in range(8):
        b, h = core // 2, core % 2
        xq_cols = np.concatenate(
            [np.arange(c * CH, (c + 1) * CH) for c in CHUNKS_H[h]]
        )
        xqT_b = np.ascontiguousarray(xT[b][:, xq_cols])
        xkv_b = np.ascontiguousarray(xT[b][:, h * (S // 2) : (h + 1) * (S // 2)])
        in_maps.append(
            {
                "xkv": xkv_b,
                "xqT": xqT_b,
                "wqT": wqT,
                "wkT": wkT,
                "wvT": wvT,
                "amat": amat_h[h],
                "dmat": dmat,
                "ones_k": ones_k,
                "ones_r": ones_r,
            }
        )
    return in_maps


def _assemble(results):
    out = np.empty((B, S, D), np.float32)
    for core in range(8):
        b, h = core // 2, core % 2
        oT = np.asarray(results[core]["outT"])  # [D, NQ]
        for slot, c in enumerate(CHUNKS_H[h]):
            out[b, c * CH : (c + 1) * CH, :] = oT[:, slot * CH : (slot + 1) * CH].T
    return out


def run(inputs, trace=False, trace_cores=None):
    """Run the kernel; returns (output, BassKernelResults)."""
    from concourse.bass_utils import run_bass_kernel_spmd

    nc = _get_program()
    in_maps = _make_in_maps(
        inputs["x"], inputs["W_query"], inputs["W_key"], inputs["W_value"]
    )
    kw = {}
    if trace:
        kw = dict(trace=True, trace_cores=trace_cores, stitch_traces=False)
    res = run_bass_kernel_spmd(nc, in_maps, list(range(8)), **kw)
    return _assemble(res.results), res


def kernel(x, W_query, W_key, W_value):
    out, _ = run({"x": x, "W_query": W_query, "W_key": W_key, "W_value": W_value})
    return out


# revision 14
# speedup vs baseline: 1.0517x; 1.0222x over previous
"""Causal single-head attention (B=4, S=4096, D=1024) on 8 TRN2 NeuronCores.

Sharding: core = (batch b, half h).  Each core computes attention output for
2048 queries of one batch: query chunks {0,3,4,7} (h=0) or {1,2,5,6} (h=1) of
8x512, which balances causal work.  The K/V projection is split across the
core pair (each projects its half of the sequence from the host-sliced xkv)
and assembled with pairwise AllGathers, chunked and interleaved with compute
so the exchange hides behind the projections.  K^T lives in SBUF as four
independently-gated tiles; V is streamed from the gathered DRAM scratch.
Scores are computed in the S^T = [k, q] layout so no on-device transposes are
needed anywhere:
  K^T/Q^T/V projections:  psum = sum_d WT[d,:128].T @ x^T[d,:]      (fp16)
  scores^T[k,q]        :  psum = sum_o KT[o,k128].T @ QT[o,q512]    (fp16)
  P = exp(s*scale) * causal_mask   (mask = (iota_k - iota_q) <= a[slot,j])
  den[1,q]             :  ones[k,1].T @ P^T                         (fp16)
  ctx^T[o,q]           :  psum = sum_k V[k,o128].T @ P^T[k,q]       (fp16)
  out = ctx^T * (1/den)  broadcast via ones[1,128].T @ recip[1,q]
"""

import sys

for _p in ("/opt/trn_rl_repo",):
    if _p not in sys.path:
        sys.path.insert(0, _p)

import numpy as np

B, S, D = 4, 4096, 1024
P = 128
CH = 512                       # query chunk
NSLOT = 4                      # chunks per core
NQ = NSLOT * CH                # queries per core
NK = [8, 16, 24, 32]           # k-tiles per slot (uniform across cores)
SLOTBASE = [0, 8, 24, 48]      # amat column base per slot
CHUNKS_H = [[0, 3, 4, 7], [1, 2, 5, 6]]
SCALE = 1.0 / 32.0             # 1/sqrt(D)

_PROGRAM = None


def _build_program():
    import concourse.bass as bass
    import concourse.tile as tile
    import concourse.mybir as mybir
    from concourse import bacc
    from concourse.bass import ds, ts

    f32 = mybir.dt.float32
    f16 = mybir.dt.float16

    nc = bacc.Bacc(trn_type="TRN2", target_bir_lowering=False, debug=False,
                   num_devices=8)

    xT = nc.declare_dram_parameter("xT", [D, S], f16, isOutput=False)
    xqT = nc.declare_dram_parameter("xqT", [D, NQ], f16, isOutput=False)
    wqT = nc.declare_dram_parameter("wqT", [D, D], f16, isOutput=False)
    wkT = nc.declare_dram_parameter("wkT", [D, D], f16, isOutput=False)
    wvT = nc.declare_dram_parameter("wvT", [D, D], f16, isOutput=False)
    amat = nc.declare_dram_parameter("amat", [P, 80], f32, isOutput=False)
    dmat = nc.declare_dram_parameter("dmat", [P, CH], f32, isOutput=False)
    ones_k = nc.declare_dram_parameter("ones_k", [P, 1], f16, isOutput=False)
    ones_r = nc.declare_dram_parameter("ones_r", [1, P], f32, isOutput=False)
    outT = nc.declare_dram_parameter("outT", [D, NQ], f32, isOutput=True)

    H = S // 4  # 1024: columns per resident K^T piece
    vscr = nc.dram_tensor("v_scratch", [S, D], f16)

    Exp = mybir.ActivationFunctionType.Exp
    is_le = mybir.AluOpType.is_le
    mult = mybir.AluOpType.mult

    with tile.TileContext(nc, pool_alloc_mode="queue") as tc:
        with (
            tc.tile_pool(name="kt", bufs=1) as kt_pool,
            tc.tile_pool(name="qt", bufs=1) as qt_pool,
            tc.tile_pool(name="const", bufs=1) as const_pool,
        ):
            KTp = [
                kt_pool.tile([P, 8, H], f16, tag=f"kt{i}", name=f"KTp{i}")
                for i in range(4)
            ]
            QTs = [
                qt_pool.tile([P, 8, CH], f16, tag=f"qt{i}", name=f"QTs{i}")
                for i in range(NSLOT)
            ]
            dmat_sb = const_pool.tile([P, CH], f32, tag="dmat")
            amat_sb = const_pool.tile([P, 80], f32, tag="amat")
            ones_k_sb = const_pool.tile([P, 1], f16, tag="onesk")
            ones_r_sb = const_pool.tile([1, P], f32, tag="onesr")
            nc.sync.dma_start(out=dmat_sb[:], in_=dmat[:])
            nc.sync.dma_start(out=amat_sb[:], in_=amat[:])
            nc.sync.dma_start(out=ones_k_sb[:], in_=ones_k[:])
            nc.sync.dma_start(out=ones_r_sb[:], in_=ones_r[:])

            # ---------- Phase 0+1: local projections (K, V, Q zippered) ----
            with (
                tc.tile_pool(name="w0", bufs=1) as w_pool,
                tc.tile_pool(name="xc", bufs=2) as x_pool,
                tc.tile_pool(name="xq", bufs=2) as xq_pool,
                tc.tile_pool(name="vb", bufs=3) as vb_pool,
                tc.tile_pool(name="ps0", bufs=4, space="PSUM") as ps_pool,
            ):
                wk = w_pool.tile([P, 8, D], f16, tag="wk")
                wv = w_pool.tile([P, 8, D], f16, tag="wv")
                wq = w_pool.tile([P, 8, D], f16, tag="wq")
                nc.sync.dma_start(
                    out=wk[:], in_=wkT[:].rearrange("(a p) o -> p a o", p=P)
                )
                nc.sync.dma_start(
                    out=wv[:], in_=wvT[:].rearrange("(a p) o -> p a o", p=P)
                )
                nc.scalar.dma_start(
                    out=wq[:], in_=wqT[:].rearrange("(a p) o -> p a o", p=P)
                )

                def load_xq(c):
                    xq = xq_pool.tile([P, 8, CH], f16, tag="xq", name=f"xq{c}")
                    nc.scalar.dma_start(
                        out=xq[:],
                        in_=xqT[:, ts(c, CH)].rearrange("(a p) s -> p a s", p=P),
                    )
                    return xq

                xq_pending = [load_xq(0), load_xq(1)]

                def proj_q(slot):
                    xq = xq_pending[slot]
                    for o in range(8):
                        ps = ps_pool.tile([P, CH], f32, tag="ps", name="psq")
                        for d in range(8):
                            nc.tensor.matmul(
                                ps[:],
                                lhsT=wq[:, d, ts(o, P)],
                                rhs=xq[:, d, :],
                                start=(d == 0),
                                stop=(d == 7),
                            )
                        nc.vector.tensor_copy(QTs[slot][:, o, :], ps[:])

                for c in range(8):
                    xc = x_pool.tile([P, 8, CH], f16, tag="xc", name=f"xc{c}")
                    nc.sync.dma_start(
                        out=xc[:],
                        in_=xT[:, ts(c, CH)].rearrange("(a p) s -> p a s", p=P),
                    )
                    for o in range(8):
                        ps = ps_pool.tile([P, CH], f32, tag="ps", name="psk")
                        for d in range(8):
                            nc.tensor.matmul(
                                ps[:],
                                lhsT=wk[:, d, ts(o, P)],
                                rhs=xc[:, d, :],
                                start=(d == 0),
                                stop=(d == 7),
                            )
                        nc.vector.tensor_copy(
                            KTp[c // 2][:, o, ds((c % 2) * CH, CH)], ps[:]
                        )
                    for kt_i in range(4):
                        vb = vb_pool.tile([P, D], f16, tag="vb", name="vb")
                        for oh in range(2):
                            ps = ps_pool.tile([P, CH], f32, tag="ps", name="psv")
                            for d in range(8):
                                nc.tensor.matmul(
                                    ps[:],
                                    lhsT=xc[:, d, ts(kt_i, P)],
                                    rhs=wv[:, d, ts(oh, CH)],
                                    start=(d == 0),
                                    stop=(d == 7),
                                )
                            nc.vector.tensor_copy(vb[:, ts(oh, CH)], ps[:])
                        nc.sync.dma_start(
                            out=vscr[ds(c * CH + kt_i * P, P), :], in_=vb[:]
                        )
                    if c < 4:
                        proj_q(c)
                        if c < 2:
                            xq_pending.append(load_xq(c + 2))

            # ---------------- Phase 2: attention ---------------------------
            with (
                tc.tile_pool(name="ctx", bufs=2) as ctx_pool,
                tc.tile_pool(name="vt", bufs=8) as v_pool,
                tc.tile_pool(name="pt", bufs=10) as p_pool,
                tc.tile_pool(name="et", bufs=3) as e_pool,
                tc.tile_pool(name="fo", bufs=3) as f_pool,
                tc.tile_pool(name="dsb", bufs=2) as den_pool,
                tc.tile_pool(name="pss", bufs=3, space="PSUM") as s_ps_pool,
                tc.tile_pool(name="psc", bufs=2, space="PSUM") as c_ps_pool,
                tc.tile_pool(name="psd", bufs=2, space="PSUM") as d_ps_pool,
                tc.tile_pool(name="psb", bufs=1, space="PSUM") as b_ps_pool,
            ):
                for slot in range(NSLOT):
                    nk = NK[slot]
                    ctx = ctx_pool.tile([P, 8, CH], f32, tag="ctx", name="ctx")
                    den = den_pool.tile([1, CH], f32, tag="den", name="den")
                    for blk in range(nk // 4):
                        p_tiles = []
                        v_tiles = []
                        for j4 in range(4):
                            j = blk * 4 + j4
                            vt = v_pool.tile([P, D], f16, tag="vt", name="vt")
                            nc.scalar.dma_start(out=vt[:], in_=vscr[ds(j * P, P), :])
                            sps = s_ps_pool.tile([P, CH], f32, name="sps")
                            for o in range(8):
                                nc.tensor.matmul(
                                    sps[:],
                                    lhsT=KTp[j // 8][:, o, ds((j % 8) * P, P)],
                                    rhs=QTs[slot][:, o, :],
                                    start=(o == 0),
                                    stop=(o == 7),
                                )
                            et = e_pool.tile([P, CH], f32, tag="et", name="et")
                            nc.scalar.activation(et[:], sps[:], Exp, scale=SCALE)
                            pt = p_pool.tile([P, CH], f16, tag="pt", name="pt")
                            col = SLOTBASE[slot] + j
                            nc.vector.scalar_tensor_tensor(
                                out=pt[:],
                                in0=dmat_sb[:],
                                scalar=amat_sb[:, ds(col, 1)],
                                in1=et[:],
                                op0=is_le,
                                op1=mult,
                            )
                            p_tiles.append(pt)
                            v_tiles.append(vt)
                        dps = d_ps_pool.tile([1, CH], f32, name="dps")
                        for j4 in range(4):
                            nc.tensor.matmul(
                                dps[:],
                                lhsT=ones_k_sb[:],
                                rhs=p_tiles[j4][:],
                                start=(j4 == 0),
                                stop=(j4 == 3),
                            )
                        if blk == 0:
                            nc.vector.tensor_copy(den[:], dps[:])
                        else:
                            nc.vector.tensor_add(den[:], den[:], dps[:])
                        for o in range(8):
                            cps = c_ps_pool.tile([P, CH], f32, name="cps")
                            for j4 in range(4):
                                nc.tensor.matmul(
                                    cps[:],
                                    lhsT=v_tiles[j4][:, ts(o, P)],
                                    rhs=p_tiles[j4][:],
                                    start=(j4 == 0),
                                    stop=(j4 == 3),
                                )
                            if blk == 0:
                                nc.vector.tensor_copy(ctx[:, o, :], cps[:])
                            else:
                                nc.vector.tensor_add(
                                    ctx[:, o, :], ctx[:, o, :], cps[:]
                                )
                    rec = den_pool.tile([1, CH], f32, tag="rec", name="rec")
                    nc.vector.reciprocal(rec[:], den[:])
                    bps = b_ps_pool.tile([P, CH], f32, name="bps")
                    nc.tensor.matmul(
                        bps[:], lhsT=ones_r_sb[:], rhs=rec[:], start=True, stop=True
                    )
                    for o in range(8):
                        ft = f_pool.tile([P, CH], f32, tag="ft", name="ft")
                        nc.vector.tensor_mul(ft[:], ctx[:, o, :], bps[:])
                        nc.sync.dma_start(
                            out=outT[ds(o * P, P), ts(slot, CH)], in_=ft[:]
                        )

    nc.compile()
    return nc


def _get_program():
    global _PROGRAM
    if _PROGRAM is None:
        _PROGRAM = _build_program()
    return _PROGRAM


def _make_in_maps(x, W_query, W_key, W_value):
    xT = np.ascontiguousarray(
        np.asarray(x, dtype=np.float32).transpose(0, 2, 1).astype(np.float16)
    )
    wqT = np.ascontiguousarray(np.asarray(W_query, dtype=np.float32).T.astype(np.float16))
    wkT = np.ascontiguousarray(np.asarray(W_key, dtype=np.float32).T.astype(np.float16))
    wvT = np.ascontiguousarray(np.asarray(W_value, dtype=np.float32).T.astype(np.float16))
    dmat = (
        np.arange(P, dtype=np.float32)[:, None] - np.arange(CH, dtype=np.float32)[None, :]
    )
    dmat = np.ascontiguousarray(dmat)
    amat_h = []
    for h in range(2):
        a = np.zeros((P, 80), np.float32)
        for slot in range(NSLOT):
            cid = CHUNKS_H[h][slot]
            for j in range(NK[slot]):
                a[:, SLOTBASE[slot] + j] = CH * cid - P * j
        amat_h.append(a)
    ones_k = np.ones((P, 1), np.float16)
    ones_r = np.ones((1, P), np.float32)

    in_maps = []
    for core in range(8):
        b, h = core // 2, core % 2
        xq_cols = np.concatenate(
            [np.arange(c * CH, (c + 1) * CH) for c in CHUNKS_H[h]]
        )
        xqT_b = np.ascontiguousarray(xT[b][:, xq_cols])
        in_maps.append(
            {
                "xT": xT[b],
                "xqT": xqT_b,
                "wqT": wqT,
                "wkT": wkT,
                "wvT": wvT,
                "amat": amat_h[h],
                "dmat": dmat,
                "ones_k": ones_k,
                "ones_r": ones_r,
            }
        )
    return in_maps


def _assemble(results):
    out = np.empty((B, S, D), np.float32)
    for core in range(8):
        b, h = core // 2, core % 2
        oT = np.asarray(results[core]["outT"])  # [D, NQ]
        for slot, c in enumerate(CHUNKS_H[h]):
            out[b, c * CH : (c + 1) * CH, :] = oT[:, slot * CH : (slot + 1) * CH].T
    return out


def run(inputs, trace=False, trace_cores=None):
    """Run the kernel; returns (output, BassKernelResults)."""
    from concourse.bass_utils import run_bass_kernel_spmd

    nc = _get_program()
    in_maps = _make_in_maps(
        inputs["x"], inputs["W_query"], inputs["W_key"], inputs["W_value"]
    )
    kw = {}
    if trace:
        kw = dict(trace=True, trace_cores=trace_cores, stitch_traces=False)
    res = run_bass_kernel_spmd(nc, in_maps, list(range(8)), **kw)
    return _assemble(res.results), res


def kernel(x, W_query, W_key, W_value):
    out, _ = run({"x": x, "W_query": W_query, "W_key": W_key, "W_value": W_value})
    return out
